# revision 14
# baseline (speedup 1.0000x reference)
"""Trainium2 kernel for nn_Circuit_41936060678727.

The reference is a 10-qubit real-amplitude circuit (CNOT ladders + RY
rotations) applied to an amplitude-embedded batch, measured with PauliZ on
each of the 10 wires.  Every gate is linear in the state, so the whole
8-layer circuit collapses to one fixed 784x1024 matrix W (orthonormal rows)
that depends only on `params`:

    out[b, p] = sum_z (x[b] @ W)[z]^2 * sign_p(z) / sum_z (x[b] @ W)[z]^2

The division makes the pipeline scale-invariant in y = x @ W, which lets the
matmul run in fp8 with generous global scales (SW on W, SX on centered x)
that keep everything out of e4m3's subnormal range.

Device math per core (2048 samples, data-parallel over 8 cores):
    mm1: y^T [1024, 2048] = Waug^T @ xaug         fp8 DoubleRow (0.5 cyc/row)
    sq    = y^2                                    scalar+vector engines, bf16
    mm2: o^T [11, 2048]  = Zsigns^T @ sq           bf16 (1 cyc/row)
Host: out = (o[:10] / o[10])^T, concat cores.

fp8 precision recovery (sim rel err ~1.3e-2 vs 2e-2 gate):
  - x is centered (x - 0.5) so its fp8 error halves; the constant shift is
    restored by 4 "bias rows" (ones on the x side, an fp8 split of
    0.5*colsum(W) on the W side) folded into the contraction for free.
  - 784 main rows pad to 5 DoubleRow chunks of 256 rows; the 496 spare
    slots carry residual-correction rows (W - fp8(W) paired with the same
    x values) that cancel most of the W quantization error.  Chunk 3 is
    the rows-0:256 residual at e5m2 (wide exponent range) and reuses the
    chunk-0 x tile already in SBUF.
"""

import numpy as np
import ml_dtypes

N_QUBITS = 10
DIM = 1 << N_QUBITS          # 1024
N_OUT = 10
D_IN = 784
B_TOTAL = 16384
N_CORES = 8
B_CORE = B_TOTAL // N_CORES  # 2048
GROUP = 512                  # batch columns per matmul (one PSUM bank, fp32)
N_GROUPS = B_CORE // GROUP   # 4
Z_CHUNK = 128
N_ZCH = DIM // Z_CHUNK       # 8
ZCOLS = 16                   # cols 0..9 = PauliZ signs, 10 = ones, 11..15 pad
NCH = 4                      # DoubleRow K-chunks of 256 rows (1024 slots)
SW = 64.0                    # global W scale (pulls W out of e4m3 subnormals)
SX = 4.0                     # global centered-x scale
N_WARM = 20                  # PE warm-up matmuls (clock ramp + DMA prefix)

E4 = ml_dtypes.float8_e4m3
E5 = ml_dtypes.float8_e5m2


# ----------------------------------------------------------------------------
# Host-side precompute: collapse the circuit to W = M[:784, :]
# ----------------------------------------------------------------------------

def _apply_ry(S, theta, q):
    B = S.shape[0]
    left, right = 1 << q, 1 << (N_QUBITS - q - 1)
    s = S.reshape(B, left, 2, right)
    c, sn = np.cos(theta / 2), np.sin(theta / 2)
    s0 = c * s[:, :, 0] - sn * s[:, :, 1]
    s1 = sn * s[:, :, 0] + c * s[:, :, 1]
    return np.stack([s0, s1], axis=2).reshape(B, DIM)


def _apply_cnot(S, q):
    B = S.shape[0]
    left, right = 1 << q, 1 << (N_QUBITS - q - 2)
    s = S.reshape(B, left, 2, 2, right)
    s = np.concatenate([s[:, :, :1], np.flip(s[:, :, 1:], axis=3)], axis=2)
    return s.reshape(B, DIM)


def _build_W(params):
    """Circuit applied to basis rows e_0..e_783 -> W[784, 1024], fp64."""
    w = np.pi * np.tanh(params.astype(np.float64))
    S = np.zeros((D_IN, DIM), dtype=np.float64)
    S[np.arange(D_IN), np.arange(D_IN)] = 1.0
    for l in range(params.shape[0]):
        for start in (0, 1):
            for i in range(start, N_QUBITS - 1, 2):
                S = _apply_cnot(S, i)
        for i in range(N_QUBITS):
            S = _apply_ry(S, w[l, i], i)
    return S


def _build_Z():
    z = np.arange(DIM)
    Z = np.zeros((DIM, ZCOLS), dtype=np.float32)
    for p in range(N_OUT):
        Z[:, p] = 1.0 - 2.0 * ((z >> (N_QUBITS - 1 - p)) & 1)
    Z[:, N_OUT] = 1.0
    # device layout [128, 8*16]: z-chunk c rows c*128..c*128+128 at cols c*16..
    Zd = Z.reshape(N_ZCH, Z_CHUNK, ZCOLS).transpose(1, 0, 2).reshape(Z_CHUNK, -1)
    return np.ascontiguousarray(Zd)


def _q(a, t):
    return np.asarray(a, np.float32).astype(t)


def _chunk_to_tile(A):
    """[256 aug rows, n] -> [128 partitions, 2 halves, n]; slot (p, i) holds
    aug row i*128 + p.  Must match between the W and x sides (it does)."""
    n = A.shape[1]
    return np.ascontiguousarray(A.reshape(2, 128, n).transpose(1, 0, 2))


def _build_weight_operands(params):
    """Returns w4 [8, 128, 4, 2, 128] e4m3: z-chunk, partition, K-chunk,
    DR-half, z-col (partition-major, contiguous 1KB/partition per z-chunk)."""
    W = _build_W(params)                     # fp64 [784, 1024]
    Ws = W * SW
    Wh = _q(Ws, E4)                          # main fp8 weights
    Wl = Ws - Wh.astype(np.float64)          # residual
    c_s = 0.5 * W.sum(axis=0) * SW * SX      # centering bias, scaled domain
    bias = []
    r = c_s.copy()
    b = _q(r / 2, E4); bias.append(b); r -= b.astype(np.float64)
    for _ in range(3):
        b = _q(r, E4); bias.append(b); r -= b.astype(np.float64)

    # e4m3 chunks in processing order [rows 0:256, 256:512, 512:768, mixed]
    che4 = [
        Wh[0:256], Wh[256:512], Wh[512:768],
        np.concatenate([
            _q(Ws[768:784], E4),             # main tail rows 768..783
            np.stack(bias, axis=0),          # 4 bias rows (x side = ones)
            _q(Wl[0:236], E4),               # residual rows 0..235
        ], axis=0),
    ]

    w4 = np.empty((N_ZCH, 128, 4, 2 * Z_CHUNK), dtype=E4)
    for j, A in enumerate(che4):
        T = _chunk_to_tile(np.ascontiguousarray(A))      # [128, 2, 1024]
        for z in range(N_ZCH):
            blk = T[:, :, z * Z_CHUNK:(z + 1) * Z_CHUNK]  # [128, 2, 128]
            w4[z, :, j, :] = blk.reshape(128, 2 * Z_CHUNK)
    return np.ascontiguousarray(w4.reshape(N_ZCH, 128, 4, 2, 128))


def _build_x_operand(x_core):
    """x [2048, 784] f32 -> xt [128, 4 phase, 4 chunk, 2, 512] e4m3
    (partition-major, grouped by column phase so each phase's DMA reads one
    contiguous 4KB block per partition; aug chunks 0-2 main, chunk 3 =
    [tail rows 768:784, ones x4, rows 0:236])."""
    xs = (x_core.astype(np.float64) - 0.5) * SX
    xh = _q(xs, E4)                          # [2048, 784]
    xT = np.ascontiguousarray(xh.T)          # [784, 2048]
    ones = np.ones((4, B_CORE), dtype=E4)
    chunks = [
        xT[0:256], xT[256:512], xT[512:768],
        np.concatenate([xT[768:784], ones, xT[0:236]], axis=0),
    ]
    xt = np.empty((4, 128, 2, B_CORE), dtype=E4)
    for c, A in enumerate(chunks):
        xt[c] = _chunk_to_tile(np.ascontiguousarray(A))
    # [c, p, i, q*512+b] -> [p, q, c, i, b]
    xt = xt.reshape(4, 128, 2, 4, GROUP).transpose(1, 3, 0, 2, 4)
    return np.ascontiguousarray(xt)


def _round_f32r(a):
    """fp32 -> float32r encoding (e8m11, RNE): low 12 mantissa bits cleared."""
    u = np.ascontiguousarray(a, dtype=np.float32).view(np.uint32)
    keep = u & np.uint32(0xFFFFF000)
    rem = u & np.uint32(0xFFF)
    inc = (rem > 0x800) | ((rem == 0x800) & (((u >> 12) & 1) == 1))
    out = keep + (inc.astype(np.uint32) << 12)
    return out.view(np.float32)


# ----------------------------------------------------------------------------
# Bass program (identical SPMD program on all 8 cores)
# ----------------------------------------------------------------------------

_NC_CACHE = {}
TRACE = False           # test harness can flip this for profiling
LAST_RESULTS = None


def _build_bass():
    from contextlib import ExitStack

    import concourse.tile as tile
    from concourse import bacc, mybir

    f32 = mybir.dt.float32
    f32r = mybir.dt.float32r
    f8e4 = mybir.dt.float8e4
    f8e5 = mybir.dt.float8e5
    bf16 = mybir.dt.bfloat16
    DR = mybir.MatmulPerfMode.DoubleRow
    MULT = mybir.AluOpType.mult

    nc = bacc.Bacc(
        "TRN2", target_bir_lowering=False, debug=False, num_devices=N_CORES
    )
    xt_d = nc.declare_dram_parameter("xt", [128, 4, 4, 2, GROUP], f8e4, isOutput=False)
    w4_d = nc.declare_dram_parameter("w4", [N_ZCH, 128, 4, 2, 128], f8e4, isOutput=False)
    zt_d = nc.declare_dram_parameter("zt", [Z_CHUNK, N_ZCH * ZCOLS], bf16, isOutput=False)
    out_d = nc.declare_dram_parameter("out", [N_OUT + 1, B_CORE], f32, isOutput=True)

    N_PH = 4                      # column phases of one 512-col group each

    with ExitStack() as ctx:
        tc = ctx.enter_context(tile.TileContext(nc))
        wpool = ctx.enter_context(tc.tile_pool(name="w", bufs=1))
        xpool = ctx.enter_context(tc.tile_pool(name="x", bufs=1))
        zpool = ctx.enter_context(tc.tile_pool(name="z", bufs=1))
        sqpool = ctx.enter_context(tc.tile_pool(name="sq", bufs=4))
        opool = ctx.enter_context(tc.tile_pool(name="osb", bufs=1))
        redpool = ctx.enter_context(tc.tile_pool(name="red", bufs=2))
        pypool = ctx.enter_context(tc.tile_pool(name="py", bufs=2, space="PSUM"))
        popool = ctx.enter_context(tc.tile_pool(name="po", bufs=2, space="PSUM"))

        # PE pre-warm: K=128 bf16 matmuls with the array fully lit -- the
        # HAM/DVFS clock ramps on real array activity.  memset on gpsimd so
        # warm starts right after the entry barrier; the warm window covers
        # the critical input DMAs (w z0/z1 + x phase 0).
        warm_in = opool.tile([128, 384], bf16, name="warm_in")
        nc.gpsimd.memset(warm_in[:], 1.0)
        warm_ps = pypool.tile([128, 2, GROUP], f32, tag="py", name="warm_ps")
        for _ in range(N_WARM):
            nc.tensor.matmul(
                warm_ps[:, 0, 0:256],
                lhsT=warm_in[:, 0:128],
                rhs=warm_in[:, 128:384],
                start=True,
                stop=True,
                skip_group_check=True,
            )

        # Input DMAs spread over four engine queues (each DGE queue streams
        # ~70 GB/s, so the 3.1 MB of inputs need the parallelism) and ordered
        # so the phase-0 critical set (w z0/z1, x phase-0 chunks) lands
        # before the PE warm-up ends.
        z_sb = zpool.tile([Z_CHUNK, N_ZCH * ZCOLS], bf16)
        w_sb = [None] * N_ZCH
        x_sb = [None] * N_PH

        def load_w(z, eng):
            t = wpool.tile([128, 4, 2, 128], f8e4, tag=f"w{z}", name=f"w{z}")
            eng.dma_start(t[:], w4_d[z])
            w_sb[z] = t

        def x_tile(q):
            if x_sb[q] is None:
                x_sb[q] = xpool.tile(
                    [128, 4, 2, GROUP], f8e4, tag=f"xq{q}", name=f"xq{q}"
                )
            return x_sb[q]

        def load_x(q, c0, c1, eng):
            t = x_tile(q)
            eng.dma_start(t[:, c0:c1], xt_d[:, q, c0:c1])

        # Only sync/gpsimd/scalar have DGE queues.  Ordered so w z0/z1 and
        # the four x phase-0 chunks land by the end of the PE warm-up, and
        # each later w z / x phase arrives before its consumer.
        nc.sync.dma_start(z_sb[:], zt_d[:, :])
        load_w(0, nc.sync)
        load_w(1, nc.gpsimd)
        load_x(0, 0, 1, nc.scalar)
        load_x(0, 1, 2, nc.scalar)
        load_x(0, 2, 3, nc.sync)
        load_x(0, 3, 4, nc.gpsimd)
        load_w(2, nc.sync)
        load_w(3, nc.gpsimd)
        load_w(4, nc.sync)
        load_w(5, nc.gpsimd)
        load_w(6, nc.sync)
        load_w(7, nc.gpsimd)
        load_x(1, 0, 2, nc.scalar)
        load_x(1, 2, 4, nc.scalar)
        load_x(2, 0, 2, nc.scalar)
        load_x(2, 2, 4, nc.scalar)
        load_x(3, 0, 2, nc.sync)
        load_x(3, 2, 4, nc.gpsimd)

        out_sb = opool.tile([N_OUT + 1, B_CORE], f32)

        # Per phase: mm1 in four z-pairs (py [128, 2, 512] spans two PSUM
        # banks, one ACTIVATE squares both z-chunks); the sign contraction
        # mm2 runs as two rounds of four column-tiled matmuls (M=11 occupies
        # one 32-wide column group; 4 concurrent tiles ~ one matmul's time).
        # Round 1 (z0-3) interleaves after pair 2; round 2 (z4-7) lands in
        # the next phase after pair 1, so it never waits on squares.
        sq_t = [None] * 4            # sq tiles by pair slot
        po_t = [None] * N_PH
        pend = []                    # deferred mm2 round-2 + reduction

        def mm1_pair(h, p):
            py = pypool.tile(
                [Z_CHUNK, 2, GROUP], f32, tag="py", name=f"py_{h}_{p}"
            )
            for zi in range(2):
                for ci in range(4):
                    nc.tensor.matmul(
                        py[:, zi],
                        lhsT=w_sb[2 * p + zi][:, ci],
                        rhs=x_sb[h][:, ci],
                        start=(ci == 0),
                        stop=(ci == NCH - 1),
                        perf_mode=DR,
                        skip_group_check=True,
                    )
            sq = sqpool.tile(
                [Z_CHUNK, 2, GROUP], bf16, tag="sq", name=f"sq_{h}_{p}"
            )
            if h == N_PH - 1 and p == 3:
                # tail: split the last pair's square so only one 512-col
                # activate remains after the final matmul
                nc.scalar.square(sq[:, 0], py[:, 0])
                nc.scalar.square(sq[:, 1], py[:, 1])
            else:
                nc.scalar.square(sq[:], py[:])
            sq_t[p] = sq

        def mm2_round(h, r, start, stop):
            po = po_t[h]
            for j in range(4):
                z = 4 * r + j
                sq = sq_t[(z // 2) % 4]
                nc.tensor.matmul(
                    po[32 * j:32 * j + N_OUT + 1, :],
                    lhsT=z_sb[:, z * ZCOLS:z * ZCOLS + N_OUT + 1],
                    rhs=sq[:, z % 2],
                    start=start,
                    stop=stop,
                    skip_group_check=True,
                    tile_position=(0, 32 * j),
                )

        def reduce_and_store(h):
            po = po_t[h]
            off = h * GROUP
            t1 = redpool.tile([N_OUT + 1, GROUP], f32, tag="ra", name=f"ra{h%2}")
            t2 = redpool.tile([N_OUT + 1, GROUP], f32, tag="rb", name=f"rb{h%2}")
            nc.vector.tensor_copy(t1[:], po[0:N_OUT + 1, :])
            nc.vector.tensor_add(t2[:], t1[:], po[32:32 + N_OUT + 1, :])
            nc.vector.tensor_add(t1[:], t2[:], po[64:64 + N_OUT + 1, :])
            nc.vector.tensor_add(
                out_sb[:, off:off + GROUP], t1[:], po[96:96 + N_OUT + 1, :]
            )

        for h in range(N_PH):
            po_t[h] = popool.tile(
                [128, GROUP], f32, tag="po", name=f"po_{h}"
            )
            mm1_pair(h, 0)
            mm1_pair(h, 1)
            for fn in pend:
                fn()
            pend = []
            mm1_pair(h, 2)
            mm2_round(h, 0, True, False)
            mm1_pair(h, 3)

            def deferred(h=h):
                mm2_round(h, 1, False, True)
                reduce_and_store(h)

            pend.append(deferred)
        for fn in pend:
            fn()

        # output DMAs: early columns stream out mid-flight, only the last
        # 512 columns sit on the tail
        nc.sync.dma_start(out_d[:, 0:1024], out_sb[:, 0:1024])
        nc.sync.dma_start(out_d[:, 1024:1536], out_sb[:, 1024:1536])
        nc.sync.dma_start(out_d[:, 1536:2048], out_sb[:, 1536:2048])

    nc.finalize()
    return nc


def _get_nc():
    if "nc" not in _NC_CACHE:
        _NC_CACHE["nc"] = _build_bass()
    return _NC_CACHE["nc"]


# ----------------------------------------------------------------------------
# Entry point
# ----------------------------------------------------------------------------

def kernel(input, params):
    global LAST_RESULTS
    from concourse.bass_utils import run_bass_kernel_spmd

    x = np.ascontiguousarray(np.asarray(input, dtype=np.float32))
    p = np.asarray(params, dtype=np.float32)

    w4 = _build_weight_operands(p)
    Z = _build_Z().astype(ml_dtypes.bfloat16)   # signs/ones: exact in bf16

    nc = _get_nc()
    in_maps = []
    for c in range(N_CORES):
        xt = _build_x_operand(x[c * B_CORE:(c + 1) * B_CORE])
        in_maps.append({"xt": xt, "w4": w4, "zt": Z})

    res = run_bass_kernel_spmd(nc, in_maps, list(range(N_CORES)), trace=TRACE)
    LAST_RESULTS = res

    outs = []
    for c in range(N_CORES):
        o = res.results[c]["out"]                 # [11, 2048]
        outs.append((o[:N_OUT] / o[N_OUT:N_OUT + 1]).T)
    return np.ascontiguousarray(np.concatenate(outs, axis=0).astype(np.float32))



# revision 17
# speedup vs baseline: 1.0557x; 1.0557x over previous
"""Trainium2 kernel for nn_Circuit_41936060678727.

The reference is a 10-qubit real-amplitude circuit (CNOT ladders + RY
rotations) applied to an amplitude-embedded batch, measured with PauliZ on
each of the 10 wires.  Every gate is linear in the state, so the whole
8-layer circuit collapses to one fixed 784x1024 matrix W (orthonormal rows)
that depends only on `params`:

    out[b, p] = sum_z (x[b] @ W)[z]^2 * sign_p(z) / sum_z (x[b] @ W)[z]^2

The division makes the pipeline scale-invariant in y = x @ W, which lets the
matmul run in fp8 with generous global scales (SW on W, SX on centered x)
that keep everything out of e4m3's subnormal range.

Device math per core (2048 samples, data-parallel over 8 cores):
    mm1: y^T [1024, 2048] = Waug^T @ xaug         fp8 DoubleRow (0.5 cyc/row)
    sq    = y^2                                    scalar+vector engines, bf16
    mm2: o^T [11, 2048]  = Zsigns^T @ sq           bf16 (1 cyc/row)
Host: out = (o[:10] / o[10])^T, concat cores.

fp8 precision recovery (sim rel err ~1.3e-2 vs 2e-2 gate):
  - x is centered (x - 0.5) so its fp8 error halves; the constant shift is
    restored by 4 "bias rows" (ones on the x side, an fp8 split of
    0.5*colsum(W) on the W side) folded into the contraction for free.
  - 784 main rows pad to 5 DoubleRow chunks of 256 rows; the 496 spare
    slots carry residual-correction rows (W - fp8(W) paired with the same
    x values) that cancel most of the W quantization error.  Chunk 3 is
    the rows-0:256 residual at e5m2 (wide exponent range) and reuses the
    chunk-0 x tile already in SBUF.
"""

import numpy as np
import ml_dtypes

N_QUBITS = 10
DIM = 1 << N_QUBITS          # 1024
N_OUT = 10
D_IN = 784
B_TOTAL = 16384
N_CORES = 8
B_CORE = B_TOTAL // N_CORES  # 2048
GROUP = 512                  # batch columns per matmul (one PSUM bank, fp32)
N_GROUPS = B_CORE // GROUP   # 4
Z_CHUNK = 128
N_ZCH = DIM // Z_CHUNK       # 8
ZCOLS = 16                   # cols 0..9 = PauliZ signs, 10 = ones, 11..15 pad
NCH = 4                      # DoubleRow K-chunks of 256 rows (1024 slots)
SW = 64.0                    # global W scale (pulls W out of e4m3 subnormals)
SX = 4.0                     # global centered-x scale
N_WARM = 26                  # PE warm-up matmuls (clock ramp + DMA prefix)

E4 = ml_dtypes.float8_e4m3
E5 = ml_dtypes.float8_e5m2


# ----------------------------------------------------------------------------
# Host-side precompute: collapse the circuit to W = M[:784, :]
# ----------------------------------------------------------------------------

def _apply_ry(S, theta, q):
    B = S.shape[0]
    left, right = 1 << q, 1 << (N_QUBITS - q - 1)
    s = S.reshape(B, left, 2, right)
    c, sn = np.cos(theta / 2), np.sin(theta / 2)
    s0 = c * s[:, :, 0] - sn * s[:, :, 1]
    s1 = sn * s[:, :, 0] + c * s[:, :, 1]
    return np.stack([s0, s1], axis=2).reshape(B, DIM)


def _apply_cnot(S, q):
    B = S.shape[0]
    left, right = 1 << q, 1 << (N_QUBITS - q - 2)
    s = S.reshape(B, left, 2, 2, right)
    s = np.concatenate([s[:, :, :1], np.flip(s[:, :, 1:], axis=3)], axis=2)
    return s.reshape(B, DIM)


def _build_W(params):
    """Circuit applied to basis rows e_0..e_783 -> W[784, 1024], fp64."""
    w = np.pi * np.tanh(params.astype(np.float64))
    S = np.zeros((D_IN, DIM), dtype=np.float64)
    S[np.arange(D_IN), np.arange(D_IN)] = 1.0
    for l in range(params.shape[0]):
        for start in (0, 1):
            for i in range(start, N_QUBITS - 1, 2):
                S = _apply_cnot(S, i)
        for i in range(N_QUBITS):
            S = _apply_ry(S, w[l, i], i)
    return S


def _build_Z():
    z = np.arange(DIM)
    Z = np.zeros((DIM, ZCOLS), dtype=np.float32)
    for p in range(N_OUT):
        Z[:, p] = 1.0 - 2.0 * ((z >> (N_QUBITS - 1 - p)) & 1)
    Z[:, N_OUT] = 1.0
    # device layout [128, 8*16]: z-chunk c rows c*128..c*128+128 at cols c*16..
    Zd = Z.reshape(N_ZCH, Z_CHUNK, ZCOLS).transpose(1, 0, 2).reshape(Z_CHUNK, -1)
    return np.ascontiguousarray(Zd)


def _q(a, t):
    return np.asarray(a, np.float32).astype(t)


def _chunk_to_tile(A):
    """[256 aug rows, n] -> [128 partitions, 2 halves, n]; slot (p, i) holds
    aug row i*128 + p.  Must match between the W and x sides (it does)."""
    n = A.shape[1]
    return np.ascontiguousarray(A.reshape(2, 128, n).transpose(1, 0, 2))


def _build_weight_operands(params):
    """Returns w4 [8, 128, 4, 2, 128] e4m3: z-chunk, partition, K-chunk,
    DR-half, z-col (partition-major, contiguous 1KB/partition per z-chunk)."""
    W = _build_W(params)                     # fp64 [784, 1024]
    Ws = W * SW
    Wh = _q(Ws, E4)                          # main fp8 weights
    Wl = Ws - Wh.astype(np.float64)          # residual
    c_s = 0.5 * W.sum(axis=0) * SW * SX      # centering bias, scaled domain
    bias = []
    r = c_s.copy()
    b = _q(r / 2, E4); bias.append(b); r -= b.astype(np.float64)
    for _ in range(3):
        b = _q(r, E4); bias.append(b); r -= b.astype(np.float64)

    # e4m3 chunks in processing order [rows 0:256, 256:512, 512:768, mixed]
    che4 = [
        Wh[0:256], Wh[256:512], Wh[512:768],
        np.concatenate([
            _q(Ws[768:784], E4),             # main tail rows 768..783
            np.stack(bias, axis=0),          # 4 bias rows (x side = ones)
            _q(Wl[0:236], E4),               # residual rows 0..235
        ], axis=0),
    ]

    w4 = np.empty((N_ZCH, 128, 4, 2 * Z_CHUNK), dtype=E4)
    for j, A in enumerate(che4):
        T = _chunk_to_tile(np.ascontiguousarray(A))      # [128, 2, 1024]
        for z in range(N_ZCH):
            blk = T[:, :, z * Z_CHUNK:(z + 1) * Z_CHUNK]  # [128, 2, 128]
            w4[z, :, j, :] = blk.reshape(128, 2 * Z_CHUNK)
    return np.ascontiguousarray(w4.reshape(N_ZCH, 128, 4, 2, 128))


def _build_x_operand(x_core):
    """x [2048, 784] f32 -> xt [128, 4 phase, 4 chunk, 2, 512] e4m3
    (partition-major, grouped by column phase so each phase's DMA reads one
    contiguous 4KB block per partition; aug chunks 0-2 main, chunk 3 =
    [tail rows 768:784, ones x4, rows 0:236])."""
    xs = (x_core.astype(np.float64) - 0.5) * SX
    xh = _q(xs, E4)                          # [2048, 784]
    xT = np.ascontiguousarray(xh.T)          # [784, 2048]
    ones = np.ones((4, B_CORE), dtype=E4)
    chunks = [
        xT[0:256], xT[256:512], xT[512:768],
        np.concatenate([xT[768:784], ones, xT[0:236]], axis=0),
    ]
    xt = np.empty((4, 128, 2, B_CORE), dtype=E4)
    for c, A in enumerate(chunks):
        xt[c] = _chunk_to_tile(np.ascontiguousarray(A))
    # [c, p, i, q*512+b] -> [p, q, c, i, b]
    xt = xt.reshape(4, 128, 2, 4, GROUP).transpose(1, 3, 0, 2, 4)
    return np.ascontiguousarray(xt)


def _round_f32r(a):
    """fp32 -> float32r encoding (e8m11, RNE): low 12 mantissa bits cleared."""
    u = np.ascontiguousarray(a, dtype=np.float32).view(np.uint32)
    keep = u & np.uint32(0xFFFFF000)
    rem = u & np.uint32(0xFFF)
    inc = (rem > 0x800) | ((rem == 0x800) & (((u >> 12) & 1) == 1))
    out = keep + (inc.astype(np.uint32) << 12)
    return out.view(np.float32)


# ----------------------------------------------------------------------------
# Bass program (identical SPMD program on all 8 cores)
# ----------------------------------------------------------------------------

_NC_CACHE = {}
TRACE = False           # test harness can flip this for profiling
LAST_RESULTS = None


def _build_bass():
    from contextlib import ExitStack

    import concourse.tile as tile
    from concourse import bacc, mybir

    f32 = mybir.dt.float32
    f32r = mybir.dt.float32r
    f8e4 = mybir.dt.float8e4
    f8e5 = mybir.dt.float8e5
    bf16 = mybir.dt.bfloat16
    DR = mybir.MatmulPerfMode.DoubleRow
    MULT = mybir.AluOpType.mult

    nc = bacc.Bacc(
        "TRN2", target_bir_lowering=False, debug=False, num_devices=N_CORES
    )
    xt_d = nc.declare_dram_parameter("xt", [128, 4, 4, 2, GROUP], f8e4, isOutput=False)
    w4_d = nc.declare_dram_parameter("w4", [N_ZCH, 128, 4, 2, 128], f8e4, isOutput=False)
    zt_d = nc.declare_dram_parameter("zt", [Z_CHUNK, N_ZCH * ZCOLS], bf16, isOutput=False)
    out_d = nc.declare_dram_parameter("out", [N_OUT + 1, B_CORE], f32, isOutput=True)

    N_PH = 4                      # column phases of one 512-col group each

    with ExitStack() as ctx:
        tc = ctx.enter_context(tile.TileContext(nc))
        wpool = ctx.enter_context(tc.tile_pool(name="w", bufs=1))
        xpool = ctx.enter_context(tc.tile_pool(name="x", bufs=1))
        zpool = ctx.enter_context(tc.tile_pool(name="z", bufs=1))
        sqpool = ctx.enter_context(tc.tile_pool(name="sq", bufs=4))
        opool = ctx.enter_context(tc.tile_pool(name="osb", bufs=1))
        redpool = ctx.enter_context(tc.tile_pool(name="red", bufs=2))
        pypool = ctx.enter_context(tc.tile_pool(name="py", bufs=2, space="PSUM"))
        popool = ctx.enter_context(tc.tile_pool(name="po", bufs=2, space="PSUM"))

        # PE pre-warm: K=128 bf16 matmuls with the array fully lit -- the
        # HAM/DVFS clock ramps on real array activity.  memset on gpsimd so
        # warm starts right after the entry barrier; the warm window covers
        # the critical input DMAs (w z0/z1 + x phase 0).
        warm_in = opool.tile([128, 384], bf16, name="warm_in")
        nc.gpsimd.memset(warm_in[:], 1.0)
        warm_ps = pypool.tile([128, 2, GROUP], f32, tag="py", name="warm_ps")
        for _ in range(N_WARM):
            nc.tensor.matmul(
                warm_ps[:, 0, 0:256],
                lhsT=warm_in[:, 0:128],
                rhs=warm_in[:, 128:384],
                start=True,
                stop=True,
                skip_group_check=True,
            )

        # Input DMAs spread over four engine queues (each DGE queue streams
        # ~70 GB/s, so the 3.1 MB of inputs need the parallelism) and ordered
        # so the phase-0 critical set (w z0/z1, x phase-0 chunks) lands
        # before the PE warm-up ends.
        z_sb = zpool.tile([Z_CHUNK, N_ZCH * ZCOLS], bf16)
        w_sb = [None] * N_ZCH
        x_sb = [None] * N_PH

        def load_w(z, eng):
            t = wpool.tile([128, 4, 2, 128], f8e4, tag=f"w{z}", name=f"w{z}")
            eng.dma_start(t[:], w4_d[z])
            w_sb[z] = t

        def x_tile(q):
            if x_sb[q] is None:
                x_sb[q] = xpool.tile(
                    [128, 4, 2, GROUP], f8e4, tag=f"xq{q}", name=f"xq{q}"
                )
            return x_sb[q]

        def load_x(q, c0, c1, eng):
            t = x_tile(q)
            eng.dma_start(t[:, c0:c1], xt_d[:, q, c0:c1])

        # Only sync/gpsimd/scalar have DGE queues (~70 GB/s each).  Ordered
        # so w z0/z1 and the four x phase-0 chunks land by the end of the PE
        # warm-up, then each later w z-pair / x phase arrives just before
        # its consumer (a single late DMA stalls the PE *and* drops the HAM
        # clock back to 1.2 GHz for several microseconds).
        nc.sync.dma_start(z_sb[:], zt_d[:, :])
        load_w(0, nc.sync)
        load_w(1, nc.gpsimd)
        load_x(0, 0, 1, nc.scalar)
        load_x(0, 2, 3, nc.sync)
        load_x(0, 3, 4, nc.gpsimd)
        load_x(0, 1, 2, nc.scalar)
        load_w(2, nc.sync)
        load_w(3, nc.gpsimd)
        load_w(4, nc.sync)
        load_w(5, nc.gpsimd)
        load_w(6, nc.sync)
        load_w(7, nc.gpsimd)
        load_x(1, 0, 2, nc.scalar)
        load_x(1, 2, 4, nc.scalar)
        load_x(2, 0, 2, nc.scalar)
        load_x(2, 2, 4, nc.scalar)
        load_x(3, 0, 2, nc.sync)
        load_x(3, 2, 4, nc.gpsimd)

        out_sb = opool.tile([N_OUT + 1, B_CORE], f32)

        # Per phase: mm1 in four z-pairs (py [128, 2, 512] spans two PSUM
        # banks, one ACTIVATE squares both z-chunks); the sign contraction
        # mm2 runs as two rounds of four column-tiled matmuls (M=11 occupies
        # one 32-wide column group; 4 concurrent tiles ~ one matmul's time).
        # Round 1 (z0-3) interleaves after pair 2; round 2 (z4-7) lands in
        # the next phase after pair 1, so it never waits on squares.
        sq_t = [None] * 4            # sq tiles by pair slot
        po_t = [None] * N_PH
        pend = []                    # deferred mm2 round-2 + reduction

        def mm1_pair(h, p):
            py = pypool.tile(
                [Z_CHUNK, 2, GROUP], f32, tag="py", name=f"py_{h}_{p}"
            )
            for zi in range(2):
                for ci in range(4):
                    nc.tensor.matmul(
                        py[:, zi],
                        lhsT=w_sb[2 * p + zi][:, ci],
                        rhs=x_sb[h][:, ci],
                        start=(ci == 0),
                        stop=(ci == NCH - 1),
                        perf_mode=DR,
                        skip_group_check=True,
                    )
            sq = sqpool.tile(
                [Z_CHUNK, 2, GROUP], bf16, tag="sq", name=f"sq_{h}_{p}"
            )
            if h == N_PH - 1 and p == 3:
                # tail: split the last pair's square so only one 512-col
                # activate remains after the final matmul
                nc.scalar.square(sq[:, 0], py[:, 0])
                nc.scalar.square(sq[:, 1], py[:, 1])
            else:
                nc.scalar.square(sq[:], py[:])
            sq_t[p] = sq

        def mm2_round(h, zs, start, stop):
            po = po_t[h]
            for j, z in enumerate(zs):
                sq = sq_t[(z // 2) % 4]
                nc.tensor.matmul(
                    po[32 * j:32 * j + N_OUT + 1, :],
                    lhsT=z_sb[:, z * ZCOLS:z * ZCOLS + N_OUT + 1],
                    rhs=sq[:, z % 2],
                    start=start,
                    stop=stop,
                    skip_group_check=True,
                    tile_position=(0, 32 * j),
                )

        def reduce_and_store(h, ngroups):
            po = po_t[h]
            off = h * GROUP
            t1 = redpool.tile([N_OUT + 1, GROUP], f32, tag="ra", name=f"ra{h%2}")
            t2 = redpool.tile([N_OUT + 1, GROUP], f32, tag="rb", name=f"rb{h%2}")
            if ngroups == 4:
                nc.vector.tensor_copy(t1[:], po[0:N_OUT + 1, :])
                nc.vector.tensor_add(t2[:], t1[:], po[32:32 + N_OUT + 1, :])
                nc.vector.tensor_add(t1[:], t2[:], po[64:64 + N_OUT + 1, :])
                nc.vector.tensor_add(
                    out_sb[:, off:off + GROUP], t1[:], po[96:96 + N_OUT + 1, :]
                )
            else:
                nc.vector.tensor_copy(t1[:], po[0:N_OUT + 1, :])
                nc.vector.tensor_add(
                    out_sb[:, off:off + GROUP], t1[:], po[32:32 + N_OUT + 1, :]
                )

        LAST = N_PH - 1
        for h in range(N_PH):
            po_t[h] = popool.tile(
                [128, GROUP], f32, tag="po", name=f"po_{h}"
            )
            mm1_pair(h, 0)
            mm1_pair(h, 1)
            for fn in pend:
                fn()
            pend = []
            mm1_pair(h, 2)
            if h < LAST:
                mm2_round(h, (0, 1, 2, 3), True, False)
            else:
                # last phase: two column groups so the tail reduction is a
                # single copy+add instead of a 4-deep chain
                mm2_round(h, (0, 1), True, False)
                mm2_round(h, (2, 3), False, False)
            mm1_pair(h, 3)

            def deferred(h=h):
                if h < LAST:
                    mm2_round(h, (4, 5, 6, 7), False, True)
                    reduce_and_store(h, 4)
                else:
                    mm2_round(h, (4, 5), False, False)
                    mm2_round(h, (6, 7), False, True)
                    reduce_and_store(h, 2)

            pend.append(deferred)
        for fn in pend:
            fn()

        # output DMAs: early columns stream out mid-flight, only the last
        # 512 columns sit on the tail
        nc.sync.dma_start(out_d[:, 0:1024], out_sb[:, 0:1024])
        nc.sync.dma_start(out_d[:, 1024:1536], out_sb[:, 1024:1536])
        nc.sync.dma_start(out_d[:, 1536:2048], out_sb[:, 1536:2048])

    nc.finalize()
    return nc


def _get_nc():
    if "nc" not in _NC_CACHE:
        _NC_CACHE["nc"] = _build_bass()
    return _NC_CACHE["nc"]


# ----------------------------------------------------------------------------
# Entry point
# ----------------------------------------------------------------------------

def kernel(input, params):
    global LAST_RESULTS
    from concourse.bass_utils import run_bass_kernel_spmd

    x = np.ascontiguousarray(np.asarray(input, dtype=np.float32))
    p = np.asarray(params, dtype=np.float32)

    w4 = _build_weight_operands(p)
    Z = _build_Z().astype(ml_dtypes.bfloat16)   # signs/ones: exact in bf16

    nc = _get_nc()
    in_maps = []
    for c in range(N_CORES):
        xt = _build_x_operand(x[c * B_CORE:(c + 1) * B_CORE])
        in_maps.append({"xt": xt, "w4": w4, "zt": Z})

    res = run_bass_kernel_spmd(nc, in_maps, list(range(N_CORES)), trace=TRACE)
    LAST_RESULTS = res

    outs = []
    for c in range(N_CORES):
        o = res.results[c]["out"]                 # [11, 2048]
        outs.append((o[:N_OUT] / o[N_OUT:N_OUT + 1]).T)
    return np.ascontiguousarray(np.concatenate(outs, axis=0).astype(np.float32))



# revision 21
# speedup vs baseline: 1.1112x; 1.0526x over previous
"""Trainium2 kernel for nn_Circuit_41936060678727.

The reference is a 10-qubit real-amplitude circuit (CNOT ladders + RY
rotations) applied to an amplitude-embedded batch, measured with PauliZ on
each of the 10 wires.  Every gate is linear in the state, so the whole
8-layer circuit collapses to one fixed 784x1024 matrix W (orthonormal rows)
that depends only on `params`:

    out[b, p] = sum_z (x[b] @ W)[z]^2 * sign_p(z) / sum_z (x[b] @ W)[z]^2

The division makes the pipeline scale-invariant in y = x @ W, which lets the
matmul run in fp8 with generous global scales (SW on W, SX on centered x)
that keep everything out of e4m3's subnormal range.

Device math per core (2048 samples, data-parallel over 8 cores):
    mm1: y^T [1024, 2048] = Waug^T @ xaug         fp8 DoubleRow (0.5 cyc/row)
    sq    = y^2                                    scalar+vector engines, bf16
    mm2: o^T [11, 2048]  = Zsigns^T @ sq           bf16 (1 cyc/row)
Host: out = (o[:10] / o[10])^T, concat cores.

fp8 precision recovery (sim rel err ~1.3e-2 vs 2e-2 gate):
  - x is centered (x - 0.5) so its fp8 error halves; the constant shift is
    restored by 4 "bias rows" (ones on the x side, an fp8 split of
    0.5*colsum(W) on the W side) folded into the contraction for free.
  - 784 main rows pad to 5 DoubleRow chunks of 256 rows; the 496 spare
    slots carry residual-correction rows (W - fp8(W) paired with the same
    x values) that cancel most of the W quantization error.  Chunk 3 is
    the rows-0:256 residual at e5m2 (wide exponent range) and reuses the
    chunk-0 x tile already in SBUF.
"""

import numpy as np
import ml_dtypes

N_QUBITS = 10
DIM = 1 << N_QUBITS          # 1024
N_OUT = 10
D_IN = 784
B_TOTAL = 16384
N_CORES = 8
B_CORE = B_TOTAL // N_CORES  # 2048
GROUP = 512                  # batch columns per matmul (one PSUM bank, fp32)
N_GROUPS = B_CORE // GROUP   # 4
Z_CHUNK = 128
N_ZCH = DIM // Z_CHUNK       # 8
ZCOLS = 16                   # cols 0..9 = PauliZ signs, 10 = ones, 11..15 pad
NCH = 4                      # DoubleRow K-chunks of 256 rows (1024 slots)
SW = 64.0                    # global W scale (pulls W out of e4m3 subnormals)
SX = 4.0                     # global centered-x scale
N_WARM = 24                  # PE warm-up matmuls (clock ramp + DMA prefix)

E4 = ml_dtypes.float8_e4m3
E5 = ml_dtypes.float8_e5m2


# ----------------------------------------------------------------------------
# Host-side precompute: collapse the circuit to W = M[:784, :]
# ----------------------------------------------------------------------------

def _apply_ry(S, theta, q):
    B = S.shape[0]
    left, right = 1 << q, 1 << (N_QUBITS - q - 1)
    s = S.reshape(B, left, 2, right)
    c, sn = np.cos(theta / 2), np.sin(theta / 2)
    s0 = c * s[:, :, 0] - sn * s[:, :, 1]
    s1 = sn * s[:, :, 0] + c * s[:, :, 1]
    return np.stack([s0, s1], axis=2).reshape(B, DIM)


def _apply_cnot(S, q):
    B = S.shape[0]
    left, right = 1 << q, 1 << (N_QUBITS - q - 2)
    s = S.reshape(B, left, 2, 2, right)
    s = np.concatenate([s[:, :, :1], np.flip(s[:, :, 1:], axis=3)], axis=2)
    return s.reshape(B, DIM)


def _build_W(params):
    """Circuit applied to basis rows e_0..e_783 -> W[784, 1024], fp64."""
    w = np.pi * np.tanh(params.astype(np.float64))
    S = np.zeros((D_IN, DIM), dtype=np.float64)
    S[np.arange(D_IN), np.arange(D_IN)] = 1.0
    for l in range(params.shape[0]):
        for start in (0, 1):
            for i in range(start, N_QUBITS - 1, 2):
                S = _apply_cnot(S, i)
        for i in range(N_QUBITS):
            S = _apply_ry(S, w[l, i], i)
    return S


def _build_Z():
    z = np.arange(DIM)
    Z = np.zeros((DIM, ZCOLS), dtype=np.float32)
    for p in range(N_OUT):
        Z[:, p] = 1.0 - 2.0 * ((z >> (N_QUBITS - 1 - p)) & 1)
    Z[:, N_OUT] = 1.0
    # device layout [128, 8*16]: z-chunk c rows c*128..c*128+128 at cols c*16..
    Zd = Z.reshape(N_ZCH, Z_CHUNK, ZCOLS).transpose(1, 0, 2).reshape(Z_CHUNK, -1)
    return np.ascontiguousarray(Zd)


def _q(a, t):
    return np.asarray(a, np.float32).astype(t)


def _chunk_to_tile(A):
    """[256 aug rows, n] -> [128 partitions, 2 halves, n]; slot (p, i) holds
    aug row i*128 + p.  Must match between the W and x sides (it does)."""
    n = A.shape[1]
    return np.ascontiguousarray(A.reshape(2, 128, n).transpose(1, 0, 2))


def _build_weight_operands(params):
    """Returns w4 [8, 128, 4, 2, 128] e4m3: z-chunk, partition, K-chunk,
    DR-half, z-col (partition-major, contiguous 1KB/partition per z-chunk)."""
    W = _build_W(params)                     # fp64 [784, 1024]
    Ws = W * SW
    Wh = _q(Ws, E4)                          # main fp8 weights
    Wl = Ws - Wh.astype(np.float64)          # residual
    c_s = 0.5 * W.sum(axis=0) * SW * SX      # centering bias, scaled domain
    bias = []
    r = c_s.copy()
    b = _q(r / 2, E4); bias.append(b); r -= b.astype(np.float64)
    for _ in range(3):
        b = _q(r, E4); bias.append(b); r -= b.astype(np.float64)

    # e4m3 chunks in processing order [rows 0:256, 256:512, 512:768, mixed]
    che4 = [
        Wh[0:256], Wh[256:512], Wh[512:768],
        np.concatenate([
            _q(Ws[768:784], E4),             # main tail rows 768..783
            np.stack(bias, axis=0),          # 4 bias rows (x side = ones)
            _q(Wl[0:236], E4),               # residual rows 0..235
        ], axis=0),
    ]

    w4 = np.empty((N_ZCH, 128, 4, 2 * Z_CHUNK), dtype=E4)
    for j, A in enumerate(che4):
        T = _chunk_to_tile(np.ascontiguousarray(A))      # [128, 2, 1024]
        for z in range(N_ZCH):
            blk = T[:, :, z * Z_CHUNK:(z + 1) * Z_CHUNK]  # [128, 2, 128]
            w4[z, :, j, :] = blk.reshape(128, 2 * Z_CHUNK)
    return np.ascontiguousarray(w4.reshape(N_ZCH, 128, 4, 2, 128))


def _build_x_operand(x_core):
    """x [2048, 784] f32 -> xt [128, 4 phase, 4 chunk, 2, 512] e4m3
    (partition-major, grouped by column phase so each phase's DMA reads one
    contiguous 4KB block per partition; aug chunks 0-2 main, chunk 3 =
    [tail rows 768:784, ones x4, rows 0:236])."""
    xs = (x_core.astype(np.float64) - 0.5) * SX
    xh = _q(xs, E4)                          # [2048, 784]
    xT = np.ascontiguousarray(xh.T)          # [784, 2048]
    ones = np.ones((4, B_CORE), dtype=E4)
    chunks = [
        xT[0:256], xT[256:512], xT[512:768],
        np.concatenate([xT[768:784], ones, xT[0:236]], axis=0),
    ]
    xt = np.empty((4, 128, 2, B_CORE), dtype=E4)
    for c, A in enumerate(chunks):
        xt[c] = _chunk_to_tile(np.ascontiguousarray(A))
    # [c, p, i, q*512+b] -> [p, q, c, i, b]
    xt = xt.reshape(4, 128, 2, 4, GROUP).transpose(1, 3, 0, 2, 4)
    return np.ascontiguousarray(xt)


def _round_f32r(a):
    """fp32 -> float32r encoding (e8m11, RNE): low 12 mantissa bits cleared."""
    u = np.ascontiguousarray(a, dtype=np.float32).view(np.uint32)
    keep = u & np.uint32(0xFFFFF000)
    rem = u & np.uint32(0xFFF)
    inc = (rem > 0x800) | ((rem == 0x800) & (((u >> 12) & 1) == 1))
    out = keep + (inc.astype(np.uint32) << 12)
    return out.view(np.float32)


# ----------------------------------------------------------------------------
# Bass program (identical SPMD program on all 8 cores)
# ----------------------------------------------------------------------------

_NC_CACHE = {}
TRACE = False           # test harness can flip this for profiling
LAST_RESULTS = None


def _build_bass():
    from contextlib import ExitStack

    import concourse.tile as tile
    from concourse import bacc, mybir

    f32 = mybir.dt.float32
    f32r = mybir.dt.float32r
    f8e4 = mybir.dt.float8e4
    f8e5 = mybir.dt.float8e5
    bf16 = mybir.dt.bfloat16
    DR = mybir.MatmulPerfMode.DoubleRow
    MULT = mybir.AluOpType.mult

    nc = bacc.Bacc(
        "TRN2", target_bir_lowering=False, debug=False, num_devices=N_CORES
    )
    xt_d = nc.declare_dram_parameter("xt", [128, 4, 4, 2, GROUP], f8e4, isOutput=False)
    w4_d = nc.declare_dram_parameter("w4", [N_ZCH, 128, 4, 2, 128], f8e4, isOutput=False)
    zt_d = nc.declare_dram_parameter("zt", [Z_CHUNK, N_ZCH * ZCOLS], bf16, isOutput=False)
    out_d = nc.declare_dram_parameter("out", [N_OUT + 1, B_CORE], f32, isOutput=True)

    N_PH = 4                      # column phases of one 512-col group each

    with ExitStack() as ctx:
        tc = ctx.enter_context(tile.TileContext(nc))
        wpool = ctx.enter_context(tc.tile_pool(name="w", bufs=1))
        xpool = ctx.enter_context(tc.tile_pool(name="x", bufs=1))
        zpool = ctx.enter_context(tc.tile_pool(name="z", bufs=1))
        sqpool = ctx.enter_context(tc.tile_pool(name="sq", bufs=4))
        opool = ctx.enter_context(tc.tile_pool(name="osb", bufs=1))
        redpool = ctx.enter_context(tc.tile_pool(name="red", bufs=2))
        pypool = ctx.enter_context(tc.tile_pool(name="py", bufs=2, space="PSUM"))
        popool = ctx.enter_context(tc.tile_pool(name="po", bufs=2, space="PSUM"))

        # PE pre-warm: K=128 bf16 matmuls with the array fully lit -- the
        # HAM/DVFS clock ramps on real array activity.  memset on gpsimd so
        # warm starts right after the entry barrier; the warm window covers
        # the critical input DMAs (w z0/z1 + x phase 0).
        warm_in = opool.tile([128, 384], bf16, name="warm_in")
        nc.gpsimd.memset(warm_in[:], 1.0)
        warm_ps = pypool.tile([128, 2, GROUP], f32, tag="py", name="warm_ps")
        for _ in range(N_WARM):
            nc.tensor.matmul(
                warm_ps[:, 0, 0:256],
                lhsT=warm_in[:, 0:128],
                rhs=warm_in[:, 128:384],
                start=True,
                stop=True,
                skip_group_check=True,
            )

        # Input DMAs spread over four engine queues (each DGE queue streams
        # ~70 GB/s, so the 3.1 MB of inputs need the parallelism) and ordered
        # so the phase-0 critical set (w z0/z1, x phase-0 chunks) lands
        # before the PE warm-up ends.
        z_sb = zpool.tile([Z_CHUNK, N_ZCH * ZCOLS], bf16)
        w_sb = [None] * N_ZCH
        x_sb = [None] * N_PH

        def load_w(z, eng):
            t = wpool.tile([128, 4, 2, 128], f8e4, tag=f"w{z}", name=f"w{z}")
            eng.dma_start(t[:], w4_d[z])
            w_sb[z] = t

        def x_tile(q):
            if x_sb[q] is None:
                x_sb[q] = xpool.tile(
                    [128, 4, 2, GROUP], f8e4, tag=f"xq{q}", name=f"xq{q}"
                )
            return x_sb[q]

        def load_x(q, c0, c1, eng):
            t = x_tile(q)
            eng.dma_start(t[:, c0:c1], xt_d[:, q, c0:c1])

        # Only sync/gpsimd/scalar have DGE queues (~70 GB/s each).  Ordered
        # so w z0/z1 and the four x phase-0 chunks land by the end of the PE
        # warm-up, then each later w z-pair / x phase arrives just before
        # its consumer (a single late DMA stalls the PE *and* drops the HAM
        # clock back to 1.2 GHz for several microseconds).
        load_w(0, nc.sync)
        load_w(1, nc.gpsimd)
        load_x(0, 0, 1, nc.scalar)
        load_x(0, 2, 3, nc.sync)
        load_x(0, 3, 4, nc.gpsimd)
        load_x(0, 1, 2, nc.scalar)
        load_w(2, nc.sync)
        load_w(3, nc.gpsimd)
        load_w(4, nc.sync)
        load_w(5, nc.gpsimd)
        load_w(6, nc.sync)
        load_w(7, nc.gpsimd)
        nc.scalar.dma_start(z_sb[:], zt_d[:, :])
        load_x(1, 0, 2, nc.scalar)
        load_x(1, 2, 4, nc.scalar)
        load_x(2, 0, 2, nc.scalar)
        load_x(2, 2, 4, nc.scalar)
        load_x(3, 0, 2, nc.sync)
        load_x(3, 2, 4, nc.gpsimd)

        out_sb = opool.tile([N_OUT + 1, B_CORE], f32)

        # Per phase: mm1 in four z-pairs (py [128, 2, 512] spans two PSUM
        # banks, one ACTIVATE squares both z-chunks); the sign contraction
        # mm2 runs as two rounds of four column-tiled matmuls (M=11 occupies
        # one 32-wide column group; 4 concurrent tiles ~ one matmul's time).
        # Round 1 (z0-3) interleaves after pair 2; round 2 (z4-7) lands in
        # the next phase after pair 1, so it never waits on squares.
        sq_t = [None] * 4            # sq tiles by pair slot
        po_t = [None] * N_PH
        pend = []                    # deferred mm2 round-2 + reduction

        def mm1_pair(h, p):
            py = pypool.tile(
                [Z_CHUNK, 2, GROUP], f32, tag="py", name=f"py_{h}_{p}"
            )
            for zi in range(2):
                for ci in range(4):
                    nc.tensor.matmul(
                        py[:, zi],
                        lhsT=w_sb[2 * p + zi][:, ci],
                        rhs=x_sb[h][:, ci],
                        start=(ci == 0),
                        stop=(ci == NCH - 1),
                        perf_mode=DR,
                        skip_group_check=True,
                    )
            sq = sqpool.tile(
                [Z_CHUNK, 2, GROUP], bf16, tag="sq", name=f"sq_{h}_{p}"
            )
            if h == N_PH - 1 and p == 3:
                # tail: split the last pair's square so only one 512-col
                # activate remains after the final matmul
                nc.scalar.square(sq[:, 0], py[:, 0])
                nc.scalar.square(sq[:, 1], py[:, 1])
            else:
                nc.scalar.square(sq[:], py[:])
            sq_t[p] = sq

        def mm2_round(h, zs, start, stop):
            po = po_t[h]
            for z, j in zs:
                sq = sq_t[(z // 2) % 4]
                nc.tensor.matmul(
                    po[32 * j:32 * j + N_OUT + 1, :],
                    lhsT=z_sb[:, z * ZCOLS:z * ZCOLS + N_OUT + 1],
                    rhs=sq[:, z % 2],
                    start=start,
                    stop=stop,
                    skip_group_check=True,
                    tile_position=(0, 32 * j),
                )

        def reduce_and_store(h):
            po = po_t[h]
            off = h * GROUP
            t1 = redpool.tile([N_OUT + 1, GROUP], f32, tag="ra", name=f"ra{h%2}")
            t2 = redpool.tile([N_OUT + 1, GROUP], f32, tag="rb", name=f"rb{h%2}")
            nc.vector.tensor_copy(t1[:], po[0:N_OUT + 1, :])
            nc.vector.tensor_add(t2[:], t1[:], po[32:32 + N_OUT + 1, :])
            nc.vector.tensor_add(t1[:], t2[:], po[64:64 + N_OUT + 1, :])
            nc.vector.tensor_add(
                out_sb[:, off:off + GROUP], t1[:], po[96:96 + N_OUT + 1, :]
            )

        LAST = N_PH - 1
        for h in range(N_PH):
            po_t[h] = popool.tile(
                [128, GROUP], f32, tag="po", name=f"po_{h}"
            )
            mm1_pair(h, 0)
            mm1_pair(h, 1)
            for fn in pend:
                fn()
            pend = []
            mm1_pair(h, 2)
            mm1_pair(h, 3)
            # mm2 round 1 sits after pair 3 so the pair-0/1 squares have
            # ample slack; round 2 lands in the next phase after pair 1.
            if h < LAST:
                mm2_round(h, ((0, 0), (1, 1), (2, 2), (3, 3)), True, False)

                def deferred(h=h):
                    mm2_round(h, ((4, 0), (5, 1), (6, 2), (7, 3)), False, True)
                    reduce_and_store(h)

                pend.append(deferred)
            else:
                # last phase: two column groups, g0 = {z0,z2,z4} closes
                # early so its PSUM->SBUF copy overlaps the z6/z7 tail
                mm2_round(h, ((0, 0), (1, 1)), True, False)
                mm2_round(h, ((2, 0), (3, 1)), False, False)

        # tail: z4 closes g0, z5-z7 accumulate into g1
        po = po_t[LAST]
        mm2_round(LAST, ((4, 0),), False, True)
        mm2_round(LAST, ((5, 1),), False, False)
        mm2_round(LAST, ((6, 1),), False, False)
        t1 = redpool.tile([N_OUT + 1, GROUP], f32, tag="ra", name="ra_t")
        nc.vector.tensor_copy(t1[:], po[0:N_OUT + 1, :])
        mm2_round(LAST, ((7, 1),), False, True)
        off = LAST * GROUP
        nc.vector.tensor_add(
            out_sb[:, off:off + GROUP], t1[:], po[32:32 + N_OUT + 1, :]
        )

        # output DMAs: early columns stream out mid-flight, only the last
        # 512 columns sit on the tail (split across two queues)
        nc.sync.dma_start(out_d[:, 0:1024], out_sb[:, 0:1024])
        nc.sync.dma_start(out_d[:, 1024:1536], out_sb[:, 1024:1536])
        nc.sync.dma_start(out_d[:, 1536:1792], out_sb[:, 1536:1792])
        nc.gpsimd.dma_start(out_d[:, 1792:2048], out_sb[:, 1792:2048])

    nc.finalize()
    return nc


def _get_nc():
    if "nc" not in _NC_CACHE:
        _NC_CACHE["nc"] = _build_bass()
    return _NC_CACHE["nc"]


# ----------------------------------------------------------------------------
# Entry point
# ----------------------------------------------------------------------------

def kernel(input, params):
    global LAST_RESULTS
    from concourse.bass_utils import run_bass_kernel_spmd

    x = np.ascontiguousarray(np.asarray(input, dtype=np.float32))
    p = np.asarray(params, dtype=np.float32)

    w4 = _build_weight_operands(p)
    Z = _build_Z().astype(ml_dtypes.bfloat16)   # signs/ones: exact in bf16

    nc = _get_nc()
    in_maps = []
    for c in range(N_CORES):
        xt = _build_x_operand(x[c * B_CORE:(c + 1) * B_CORE])
        in_maps.append({"xt": xt, "w4": w4, "zt": Z})

    res = run_bass_kernel_spmd(nc, in_maps, list(range(N_CORES)), trace=TRACE)
    LAST_RESULTS = res

    outs = []
    for c in range(N_CORES):
        o = res.results[c]["out"]                 # [11, 2048]
        outs.append((o[:N_OUT] / o[N_OUT:N_OUT + 1]).T)
    return np.ascontiguousarray(np.concatenate(outs, axis=0).astype(np.float32))

